# revision 1
# baseline (speedup 1.0000x reference)
"""Trainium2 Bass kernel for nn_AutoIFS_89378269430151 (moe_routing).

Data-parallel over batch across 8 NeuronCores; all params replicated.
Feature-major layout on chip (features on partitions, batch on free dim),
fp32r matmuls, embedding gather via SWDGE dma_gather on a (25000, 256)
reshaped view of the table with int16 indices (idx = x>>2, quarter = x&3
selected on DVE).
"""

import os

os.environ.setdefault("JAX_PLATFORMS", "axon")

import numpy as np

# ---- problem constants (hardcoded; must match reference.py) ----
B, F, L = 16384, 32, 64
FEAT, DOM, R = 100000, 8, 16
D_IN = F * L  # 2048
NCORES = 8
BC = B // NCORES  # 2048 samples per core
NB = 512  # batch tile (free dim)
NT = BC // NB  # 4 batch tiles per core
P = 128

V4 = FEAT // 4  # 25000 rows in reshaped table
E4 = 4 * L  # 256 elements (1KB) per reshaped row
NSUB = 16  # sub-gathers per batch tile
NIDX = NB * F // NSUB  # 1024 indices per sub-gather
SLOT = NIDX // P  # 8 slots per sub-gather

_BUILT = {}
DEBUG = False
W0_BF16 = True


def build():
    if "nc" in _BUILT:
        return _BUILT["nc"]
    import concourse.bass as bass
    from concourse import bacc
    import concourse.mybir as mybir
    import concourse.tile as tile

    dt = mybir.dt
    AF = mybir.ActivationFunctionType
    OP = mybir.AluOpType
    f32, f32r, i16 = dt.float32, dt.float32r, dt.int16

    W0DT = dt.bfloat16 if W0_BF16 else dt.float32r
    nc = bacc.Bacc(None, target_bir_lowering=False, num_swdge_queues=4)

    def din(name, shape, dtype=f32):
        return nc.dram_tensor(name, shape, dtype, kind="ExternalInput")

    # ---- DRAM inputs (per core) ----
    emb4 = din("emb4", [V4, E4])  # reshaped embedding table
    idxd = din("idxd", [P, NT * NSUB * NIDX // 16], i16)  # wrapped gather idx
    qvd = din("qvd", [P, NT * P])  # quarter (x&3) per slot, f32
    dfm = din("dfm", [1, BC], f32r)  # domain ids (f32 bits, feature-major row)

    sW0 = din("sW0", [D_IN, 1024], W0DT)
    gW0 = din("gW0", [D_IN, 1024], W0DT)
    sW1 = din("sW1", [1024, 512], f32r)
    gW1 = din("gW1", [1024, 512], f32r)
    sb0 = din("sb0", [1024, 1])
    gb0 = din("gb0", [1024, 1])
    sb1 = din("sb1", [512, 1])
    gb1 = din("gb1", [512, 1])

    Dk0 = din("Dk0", [512, 256], f32r)
    Db0 = din("Db0", [256, 1])
    DA0p = din("DA0p", [512, 128], f32r)  # [i, dom*16+r]
    DB0p = din("DB0p", [128, 256], f32r)  # [dom*16+r, o]
    Dlb0p = din("Dlb0p", [8, 256], f32r)  # [dom, o]

    Tk0 = din("Tk0", [256, 128], f32r)
    TA0_0 = din("TA0_0", [256, 16], f32r)
    TA0_1 = din("TA0_1", [256, 16], f32r)
    TB0_0 = din("TB0_0", [16, 128], f32r)
    TB0_1 = din("TB0_1", [16, 128], f32r)
    bias_d0 = din("bias_d0", [128, 1])  # Tb0 + Tlb0[0]
    bias_d1 = din("bias_d1", [128, 1])  # Tb0 + Tlb0[1]
    bias_s = din("bias_s", [128, 1])  # Tb0
    bias_l0 = din("bias_l0", [128, 1])  # Tlb0[0]
    bias_l1 = din("bias_l1", [128, 1])  # Tlb0[1]

    pk1_0 = din("pk1_0", [128, 32], f32r)  # cols 0:16 TA1[0], col 16 Tk1
    pk1_1 = din("pk1_1", [128, 32], f32r)
    TA1_0 = din("TA1_0", [128, 16], f32r)
    TA1_1 = din("TA1_1", [128, 16], f32r)
    Tk1t = din("Tk1t", [128, 1], f32r)
    TB1_0 = din("TB1_0", [16, 1], f32r)
    TB1_1 = din("TB1_1", [16, 1], f32r)
    # per-partition scalar constants (value replicated on 128 partitions)
    c_d0 = din("c_d0", [128, 1])  # Tb1 + Tlb1[0]
    c_d1 = din("c_d1", [128, 1])  # Tb1 + Tlb1[1]
    c_s = din("c_s", [128, 1])  # Tb1
    c_l0 = din("c_l0", [128, 1])  # Tlb1[0]
    c_l1 = din("c_l1", [128, 1])  # Tlb1[1]

    HWp = din("HWp", [512, 8], f32r)  # [h0W | h1W]
    hb = din("hb", [8, 1])

    onesr = din("onesr", [1, 128], f32r)
    diota = din("diota", [128, 1])  # p // 16
    iota8 = din("iota8", [8, 1])
    ident = din("ident", [128, 128])
    # one-hot row-broadcast selectors
    E32 = din("E32", [32, 128], f32r)  # row 16 all-ones (broadcast ds row)
    G8sel = din("G8sel", [8, 8 * 128], f32r)  # row-broadcast for gates rows 0..7

    out = nc.dram_tensor("out", [2, BC, 128], f32, kind="ExternalOutput")
    dbg = {}
    if DEBUG:
        for nm, shp in [
            ("xT", [128, 16 * 512]), ("h0s", [128, 8 * 512]),
            ("h1s", [128, 4 * 512]), ("share", [128, 2 * 512]),
            ("lora", [128, 2 * 512]), ("ddnn", [128, 2 * 512]),
            ("gates", [8, 512]), ("mbc", [128, 512]),
            ("tdnn0", [128, 512]), ("tsl0", [128, 512]), ("tls1", [128, 512]),
        ]:
            dbg[nm] = nc.dram_tensor("dbg_" + nm, shp, f32, kind="ExternalOutput")

    with tile.TileContext(nc) as tc:
        with (
            tc.tile_pool(name="const", bufs=1) as cp,
            tc.tile_pool(name="wstream", bufs=2) as wp,
            tc.tile_pool(name="gpool", bufs=2) as gp,
            tc.tile_pool(name="selpool", bufs=2) as sp,
            tc.tile_pool(name="xt", bufs=2) as xp,
            tc.tile_pool(name="hpool", bufs=1) as hp,
            tc.tile_pool(name="task", bufs=1) as tkp,
            tc.tile_pool(name="small", bufs=2) as smp,
            tc.tile_pool(name="scratch", bufs=4) as scr,
            tc.tile_pool(name="bounce", bufs=2) as bnc,
            tc.tile_pool(name="rowb", bufs=2) as rbp,
            tc.tile_pool(name="ps", bufs=8, space="PSUM") as psp,
        ):
            def pst_(tag="ps"):
                return psp.tile([128, 512], dt.float32, space="PSUM", tag="ps",
                                name="pstile")

            # ---- resident tensors ----
            def load_const(t, shape, dtype=f32):
                s = cp.tile(shape, dtype, tag=t.name)
                nc.sync.dma_start(out=s[:], in_=t[:])
                return s

            sb0_s = load_const(sb0, [128, 8, 1])
            gb0_s = load_const(gb0, [128, 8, 1])
            sb1_s = load_const(sb1, [128, 4, 1])
            gb1_s = load_const(gb1, [128, 4, 1])
            Dk0_s = load_const(Dk0, [128, 4, 256], f32r)
            Db0_s = load_const(Db0, [128, 2, 1])
            DA0p_s = load_const(DA0p, [128, 4, 128], f32r)
            DB0p_s = load_const(DB0p, [128, 256], f32r)
            Dlb0p_s = load_const(Dlb0p, [8, 256], f32r)
            Tk0_s = load_const(Tk0, [128, 2, 128], f32r)
            TA0_0s = load_const(TA0_0, [128, 2, 16], f32r)
            TA0_1s = load_const(TA0_1, [128, 2, 16], f32r)
            TB0_0s = load_const(TB0_0, [16, 128], f32r)
            TB0_1s = load_const(TB0_1, [16, 128], f32r)
            bd0_s = load_const(bias_d0, [128, 1])
            bd1_s = load_const(bias_d1, [128, 1])
            bs_s = load_const(bias_s, [128, 1])
            bl0_s = load_const(bias_l0, [128, 1])
            bl1_s = load_const(bias_l1, [128, 1])
            pk1_0s = load_const(pk1_0, [128, 32], f32r)
            pk1_1s = load_const(pk1_1, [128, 32], f32r)
            TA1_0s = load_const(TA1_0, [128, 16], f32r)
            TA1_1s = load_const(TA1_1, [128, 16], f32r)
            Tk1_s = load_const(Tk1t, [128, 1], f32r)
            TB1_0s = load_const(TB1_0, [16, 1], f32r)
            TB1_1s = load_const(TB1_1, [16, 1], f32r)
            cd_s = [load_const(c_d0, [128, 1]), load_const(c_d1, [128, 1])]
            cs_s = load_const(c_s, [128, 1])
            cl_s = [load_const(c_l0, [128, 1]), load_const(c_l1, [128, 1])]
            HW_s = load_const(HWp, [128, 4, 8], f32r)
            hb_s = load_const(hb, [8, 1])
            ones_s = load_const(onesr, [1, 128], f32r)
            diota_s = load_const(diota, [128, 1])
            iota8_s = load_const(iota8, [8, 1])
            id_s = load_const(ident, [128, 128])
            E32_s = load_const(E32, [32, 128], f32r)
            G8_s = load_const(G8sel, [8, 8, 128], f32r)

            ICOLS = NSUB * NIDX // 16  # idx cols per tile (1024)

            def load_tile_inputs(t):
                idx_t = smp.tile([P, ICOLS], i16, tag="idx", name="idx_t")
                nc.scalar.dma_start(
                    out=idx_t[:], in_=idxd[:, t * ICOLS : (t + 1) * ICOLS]
                )
                qv_t = smp.tile([P, P], f32, tag="qv", name="qv_t")
                nc.scalar.dma_start(out=qv_t[:], in_=qvd[:, t * P : (t + 1) * P])
                dfm_t = smp.tile([1, NB], f32r, tag="dfm", name="dfm_t")
                nc.scalar.dma_start(
                    out=dfm_t[:], in_=dfm[:, t * NB : (t + 1) * NB]
                )
                return idx_t, qv_t, dfm_t

            def make_pump(idx_t, qv_t, xT_dst):
                state = {"j": 0}

                def pump(n=1):
                    for _ in range(n):
                        j = state["j"]
                        if j >= NSUB:
                            return
                        state["j"] += 1
                        g = gp.tile([P, SLOT, E4], f32, tag="g", name="g")
                        c0 = j * (NIDX // 16)
                        nc.gpsimd.dma_gather(
                            out_ap=g[:],
                            in_ap=emb4[:],
                            idxs_ap=idx_t[:, c0 : c0 + NIDX // 16],
                            num_idxs=NIDX,
                            num_idxs_reg=NIDX,
                            elem_size=E4,
                            single_packet=False,
                            queue_num=j % 4,
                        )
                        sel = sp.tile([P, SLOT, L], f32, tag="sel", name="sel")
                        qs = qv_t[:, j * SLOT : (j + 1) * SLOT]
                        nc.vector.tensor_copy(out=sel[:], in_=g[:, :, 0:L])
                        for q in range(1, 4):
                            mq = smp.tile([P, SLOT], dt.uint8, tag="mq", name="mq")
                            nc.vector.tensor_scalar(
                                out=mq[:], in0=qs, scalar1=float(q), scalar2=None,
                                op0=OP.is_equal,
                            )
                            nc.vector.copy_predicated(
                                out=sel[:],
                                mask=mq[:, :, None].to_broadcast([P, SLOT, L]),
                                data=g[:, :, q * L : (q + 1) * L],
                            )
                        cc = j // 4
                        for f2 in range(SLOT // 2):
                            pt = pst_()
                            nc.tensor.transpose(
                                out=pt[0:128, 0:128],
                                in_=sel[:, 2 * f2 : 2 * f2 + 2, :],
                                identity=id_s[:],
                            )
                            ff = (j % 4) * 4 + f2
                            nc.vector.tensor_copy(
                                out=xT_dst[:, ff, cc * 128 : (cc + 1) * 128],
                                in_=pt[0:128, 0:128],
                            )

                return pump

            def tower_l0(Wd, bias_tile, xT, pump):
                h0 = hp.tile([128, 8, 512], f32r, tag="h0", name="h0")
                for half in range(2):
                    pst = [pst_() for _ in range(4)]
                    for k in range(16):
                        wt = wp.tile([128, 512], W0DT, tag="w0", name="wt")
                        nc.sync.dma_start(
                            out=wt[:],
                            in_=Wd[k * 128 : (k + 1) * 128,
                                   half * 512 : (half + 1) * 512],
                        )
                        for m in range(4):
                            nc.tensor.matmul(
                                out=pst[m][:],
                                lhsT=wt[:, m * 128 : (m + 1) * 128],
                                rhs=xT[:, k, :],
                                start=(k == 0),
                                stop=(k == 15),
                            )
                        if k % 4 == 3:
                            pump()
                    for m in range(4):
                        mm = half * 4 + m
                        nc.scalar.activation(
                            out=h0[:, mm, :], in_=pst[m][:], func=AF.Relu,
                            bias=bias_tile[:, mm, :],
                        )
                return h0

            def tower_l1(h0, W1d, b1s, pump):
                h1 = hp.tile([128, 4, 512], f32r, tag="h1", name="h1")
                pst = [pst_() for _ in range(4)]
                for k in range(8):
                    w1t = wp.tile([128, 512], f32r, tag="w1", name="w1t")
                    nc.sync.dma_start(
                        out=w1t[:], in_=W1d[k * 128 : (k + 1) * 128, :]
                    )
                    for m in range(4):
                        nc.tensor.matmul(
                            out=pst[m][:],
                            lhsT=w1t[:, m * 128 : (m + 1) * 128],
                            rhs=h0[:, k, :],
                            start=(k == 0),
                            stop=(k == 7),
                        )
                    if k % 4 == 3:
                        pump()
                for m in range(4):
                    nc.scalar.activation(
                        out=h1[:, m, :], in_=pst[m][:], func=AF.Relu,
                        bias=b1s[:, m, :],
                    )
                return h1

            # -------- prologue: tile 0 inputs + full gather --------
            idx_c, qv_c, dfm_c = load_tile_inputs(0)
            xT_cur = xp.tile([128, 16, 512], W0DT, tag="xT", name="xT0")
            pump0 = make_pump(idx_c, qv_c, xT_cur)
            pump0(NSUB)

            for t in range(NT):
                dfm_t = dfm_c
                if t + 1 < NT:
                    idx_n, qv_n, dfm_c = load_tile_inputs(t + 1)
                    xT_next = xp.tile([128, 16, 512], W0DT, tag="xT", name="xTn")
                    pump = make_pump(idx_n, qv_n, xT_next)
                else:
                    xT_next = None
                    pump = lambda n=1: None
                xT = xT_cur

                # ================= gate tower + gates =================
                h0g = tower_l0(gW0, gb0_s, xT, pump)
                hyper = tower_l1(h0g, gW1, gb1_s, pump)
                psg = pst_()
                for k in range(4):
                    nc.tensor.matmul(
                        out=psg[0:8, :], lhsT=HW_s[:, k, :], rhs=hyper[:, k, :],
                        start=(k == 0), stop=(k == 3),
                    )
                gates = tkp.tile([8, 512], f32r, tag="gates", name="gates")
                nc.scalar.activation(
                    out=gates[:], in_=psg[0:8, :], func=AF.Sigmoid, bias=hb_s[:]
                )

                # ================= share tower =================
                h0s = tower_l0(sW0, sb0_s, xT, pump)
                h1s = tower_l1(h0s, sW1, sb1_s, pump)
                pump(NSUB)  # flush any remaining next-tile gathers

                # ================= domain layer =================
                psd = pst_()
                nc.tensor.matmul(
                    out=psd[:], lhsT=ones_s[:], rhs=dfm_t[:],
                    start=True, stop=True,
                )
                mbc = tkp.tile([128, 512], f32, tag="mbc", name="mbc")
                nc.vector.tensor_scalar(
                    out=mbc[:], in0=psd[:], scalar1=diota_s[:], scalar2=None,
                    op0=OP.is_equal,
                )
                mask8 = tkp.tile([8, 512], f32r, tag="mask8", name="mask8")
                nc.vector.tensor_scalar(
                    out=mask8[:], in0=psd[0:8, :], scalar1=iota8_s[:], scalar2=None,
                    op0=OP.is_equal,
                )
                psR = pst_()
                for k in range(4):
                    nc.tensor.matmul(
                        out=psR[:], lhsT=DA0p_s[:, k, :], rhs=h1s[:, k, :],
                        start=(k == 0), stop=(k == 3),
                    )
                rmask = tkp.tile([128, 512], f32r, tag="rmask", name="rmask")
                nc.vector.tensor_tensor(
                    out=rmask[:], in0=psR[:], in1=mbc[:], op=OP.mult
                )
                share = tkp.tile([128, 2, 512], f32r, tag="share", name="share")
                lora = tkp.tile([128, 2, 512], f32r, tag="lora", name="lora")
                ddnn = tkp.tile([128, 2, 512], f32r, tag="ddnn", name="ddnn")
                for m in range(2):
                    pss = pst_()
                    for k in range(4):
                        nc.tensor.matmul(
                            out=pss[:],
                            lhsT=Dk0_s[:, k, m * 128 : (m + 1) * 128],
                            rhs=h1s[:, k, :],
                            start=(k == 0), stop=(k == 3),
                        )
                    psl = pst_()
                    nc.tensor.matmul(
                        out=psl[:], lhsT=DB0p_s[:, m * 128 : (m + 1) * 128],
                        rhs=rmask[:], start=True, stop=False,
                    )
                    nc.tensor.matmul(
                        out=psl[:], lhsT=Dlb0p_s[:, m * 128 : (m + 1) * 128],
                        rhs=mask8[:], start=False, stop=True,
                    )
                    nc.vector.tensor_scalar(
                        out=share[:, m, :], in0=pss[:], scalar1=Db0_s[:, m, :],
                        scalar2=None, op0=OP.add,
                    )
                    nc.vector.tensor_copy(out=lora[:, m, :], in_=psl[:])
                    tmp = scr.tile([128, 512], f32, tag="ow", name="tmp")
                    nc.vector.tensor_tensor(
                        out=tmp[:], in0=share[:, m, :].bitcast(f32),
                        in1=lora[:, m, :].bitcast(f32), op=OP.add
                    )
                    nc.scalar.activation(
                        out=ddnn[:, m, :], in_=tmp[:], func=AF.Relu,
                    )

                # ================= task layer i=0 =================
                def aprod(TA_s, rhs3):
                    pa = pst_()
                    nc.tensor.matmul(out=pa[0:16, :], lhsT=TA_s[:, 0, :],
                                     rhs=rhs3[:, 0, :], start=True, stop=False)
                    nc.tensor.matmul(out=pa[0:16, :], lhsT=TA_s[:, 1, :],
                                     rhs=rhs3[:, 1, :], start=False, stop=True)
                    ab = bnc.tile([16, 512], f32r, tag="Abuf", name="ab")
                    nc.vector.tensor_copy(out=ab[:], in_=pa[0:16, :])
                    return ab

                def mm_k2(pstile, lhs_tile, rhs3, last):
                    nc.tensor.matmul(out=pstile[:], lhsT=lhs_tile[:, 0, :],
                                     rhs=rhs3[:, 0, :], start=True, stop=False)
                    nc.tensor.matmul(out=pstile[:], lhsT=lhs_tile[:, 1, :],
                                     rhs=rhs3[:, 1, :], start=False, stop=last)

                ab_d0 = aprod(TA0_0s, ddnn)
                ps_d0 = pst_()
                mm_k2(ps_d0, Tk0_s, ddnn, last=False)
                nc.tensor.matmul(out=ps_d0[:], lhsT=TB0_0s[:], rhs=ab_d0[:],
                                 start=False, stop=True)
                ab_d1 = aprod(TA0_1s, ddnn)
                ps_d1 = pst_()
                mm_k2(ps_d1, Tk0_s, ddnn, last=False)
                nc.tensor.matmul(out=ps_d1[:], lhsT=TB0_1s[:], rhs=ab_d1[:],
                                 start=False, stop=True)
                ab_s0 = aprod(TA0_0s, share)
                ps_sl0 = pst_()
                nc.tensor.matmul(out=ps_sl0[:], lhsT=TB0_0s[:], rhs=ab_s0[:],
                                 start=True, stop=True)
                ab_s1 = aprod(TA0_1s, share)
                ps_sl1 = pst_()
                nc.tensor.matmul(out=ps_sl1[:], lhsT=TB0_1s[:], rhs=ab_s1[:],
                                 start=True, stop=True)
                ab_l0 = aprod(TA0_0s, lora)
                ps_ll0 = pst_()
                nc.tensor.matmul(out=ps_ll0[:], lhsT=TB0_0s[:], rhs=ab_l0[:],
                                 start=True, stop=True)
                ab_l1 = aprod(TA0_1s, lora)
                ps_ll1 = pst_()
                nc.tensor.matmul(out=ps_ll1[:], lhsT=TB0_1s[:], rhs=ab_l1[:],
                                 start=True, stop=True)
                ps_S = pst_()
                mm_k2(ps_S, Tk0_s, share, last=True)
                ps_L = pst_()
                mm_k2(ps_L, Tk0_s, lora, last=True)

                def epi_relu(pstile, bias, dtype=f32r, tag="tk"):
                    o = tkp.tile([128, 512], dtype, tag=tag, name="epi")
                    nc.scalar.activation(out=o[:], in_=pstile[:], func=AF.Relu,
                                         bias=bias[:])
                    return o

                def epi_add(pstile, bias, dtype=f32r, tag="tk"):
                    o = tkp.tile([128, 512], dtype, tag=tag, name="epi")
                    nc.vector.tensor_scalar(out=o[:], in0=pstile[:], scalar1=bias[:],
                                            scalar2=None, op0=OP.add)
                    return o

                t_dnn0 = epi_relu(ps_d0, bd0_s, tag="tdnn0")
                t_dnn1 = epi_add(ps_d1, bd1_s, tag="tdnn1")
                t_sh0 = epi_relu(ps_S, bs_s, tag="tsh0")
                t_sh1 = epi_add(ps_S, bs_s, tag="tsh1")
                t_ls0 = epi_relu(ps_L, bs_s, f32, tag="tls0")
                t_ls1 = epi_add(ps_L, bs_s, f32, tag="tls1")
                t_lo0 = epi_relu(ps_ll0, bl0_s, tag="tlo0")
                t_lo1 = epi_add(ps_ll1, bl1_s, tag="tlo1")
                t_sl0 = epi_relu(ps_sl0, bl0_s, f32, tag="tsl0")
                t_sl1 = epi_add(ps_sl1, bl1_s, f32, tag="tsl1")

                # ================= task layer i=1 + output =================
                pk1s = [pk1_0s, pk1_1s]
                TA1s = [TA1_0s, TA1_1s]
                TB1s = [TB1_0s, TB1_1s]
                tdnns = [t_dnn0, t_dnn1]
                tshs = [t_sh0, t_sh1]
                tlos = [t_lo0, t_lo1]
                slts = [t_sl0, t_sl1]
                lsts = [t_ls0, t_ls1]

                if DEBUG and t == 0:
                    def ddump(nm, tl, rows=128):
                        nc.sync.dma_start(out=dbg[nm][:], in_=tl[0:rows].rearrange("p a b -> p (a b)") if tl[0:rows].ndim == 3 else tl[0:rows])
                    pass  # xT debug dump disabled (dtype varies)
                    nc.sync.dma_start(out=dbg["h0s"][:], in_=h0s[:].rearrange("p a b -> p (a b)").bitcast(f32))
                    nc.sync.dma_start(out=dbg["h1s"][:], in_=h1s[:].rearrange("p a b -> p (a b)").bitcast(f32))
                    nc.sync.dma_start(out=dbg["share"][:], in_=share[:].rearrange("p a b -> p (a b)").bitcast(f32))
                    nc.sync.dma_start(out=dbg["lora"][:], in_=lora[:].rearrange("p a b -> p (a b)").bitcast(f32))
                    nc.sync.dma_start(out=dbg["ddnn"][:], in_=ddnn[:].rearrange("p a b -> p (a b)").bitcast(f32))
                    nc.sync.dma_start(out=dbg["gates"][:], in_=gates[:].bitcast(f32))
                    nc.sync.dma_start(out=dbg["mbc"][:], in_=mbc[:])
                    nc.sync.dma_start(out=dbg["tdnn0"][:], in_=t_dnn0[:].bitcast(f32))
                    nc.sync.dma_start(out=dbg["tsl0"][:], in_=t_sl0[:])
                    nc.sync.dma_start(out=dbg["tls1"][:], in_=t_ls1[:])

                for tt in range(2):
                    # F block: rows 0:16 = dA1, row 16 = ds
                    pf = pst_()
                    nc.tensor.matmul(out=pf[0:32, :], lhsT=pk1s[tt][:],
                                     rhs=tdnns[tt][:], start=True, stop=True)
                    fb = bnc.tile([32, 512], f32r, tag="Fbuf", name="fb")
                    nc.vector.tensor_copy(out=fb[:], in_=pf[0:32, :])
                    # lA1
                    pl = pst_()
                    nc.tensor.matmul(out=pl[0:16, :], lhsT=TA1s[tt][:],
                                     rhs=tlos[tt][:], start=True, stop=True)
                    lb = bnc.tile([16, 512], f32r, tag="Abuf", name="lb")
                    nc.vector.tensor_copy(out=lb[:], in_=pl[0:16, :])

                    def rowbc(lhsT, rhs):
                        # (1,512) product -> SBUF row -> broadcast to 128 partitions
                        pr = pst_()
                        nc.tensor.matmul(out=pr[0:1, :], lhsT=lhsT, rhs=rhs,
                                         start=True, stop=True)
                        rb = rbp.tile([1, 512], f32r, tag="rowb", name="rb")
                        nc.vector.tensor_copy(out=rb[:], in_=pr[0:1, :])
                        pb = pst_()
                        nc.tensor.matmul(out=pb[:], lhsT=ones_s[:], rhs=rb[:],
                                         start=True, stop=True)
                        return pb

                    b_dl = rowbc(TB1s[tt][:], fb[0:16, :])
                    b_ll = rowbc(TB1s[tt][:], lb[:])
                    b_ss = rowbc(Tk1_s[:], tshs[tt][:])
                    # ds broadcast straight from fb row 16 via one-hot K=32
                    b_ds = pst_()
                    nc.tensor.matmul(out=b_ds[:], lhsT=E32_s[:], rhs=fb[:],
                                     start=True, stop=True)
                    # gates broadcasts (K=8 one-hot)
                    b_g0 = pst_()
                    nc.tensor.matmul(out=b_g0[:], lhsT=G8_s[:, 4 * tt, :],
                                     rhs=gates[:], start=True, stop=True)
                    b_g1 = pst_()
                    nc.tensor.matmul(out=b_g1[:], lhsT=G8_s[:, 4 * tt + 1, :],
                                     rhs=gates[:], start=True, stop=True)
                    b_g2 = pst_()
                    nc.tensor.matmul(out=b_g2[:], lhsT=G8_s[:, 4 * tt + 2, :],
                                     rhs=gates[:], start=True, stop=True)
                    b_g3 = pst_()
                    nc.tensor.matmul(out=b_g3[:], lhsT=G8_s[:, 4 * tt + 3, :],
                                     rhs=gates[:], start=True, stop=True)

                    # a2 = (ss + c_s) * g0 ; b2 = (ll + c_l_t) * g1
                    # w2 = ds + c_d_t + dl ; s = w2 - a2 - b2
                    a1t = scr.tile([128, 512], f32, tag="ow", name="a1t")
                    nc.vector.tensor_scalar(out=a1t[:], in0=b_ss[:],
                                            scalar1=cs_s[:], scalar2=None,
                                            op0=OP.add)
                    a2t = scr.tile([128, 512], f32, tag="ow", name="a2t")
                    nc.vector.tensor_tensor(out=a2t[:], in0=a1t[:], in1=b_g0[:],
                                            op=OP.mult)
                    b1t = scr.tile([128, 512], f32, tag="ow", name="b1t")
                    nc.vector.tensor_scalar(out=b1t[:], in0=b_ll[:],
                                            scalar1=cl_s[tt][:], scalar2=None,
                                            op0=OP.add)
                    b2t = scr.tile([128, 512], f32, tag="ow", name="b2t")
                    nc.vector.tensor_tensor(out=b2t[:], in0=b1t[:], in1=b_g1[:],
                                            op=OP.mult)
                    w1t = scr.tile([128, 512], f32, tag="ow", name="w1t")
                    nc.vector.tensor_scalar(out=w1t[:], in0=b_ds[:],
                                            scalar1=cd_s[tt][:], scalar2=None,
                                            op0=OP.add)
                    w2t = scr.tile([128, 512], f32, tag="ow", name="w2t")
                    nc.vector.tensor_tensor(out=w2t[:], in0=w1t[:], in1=b_dl[:],
                                            op=OP.add)
                    s1t = scr.tile([128, 512], f32, tag="ow", name="s1t")
                    nc.vector.tensor_tensor(out=s1t[:], in0=w2t[:], in1=a2t[:],
                                            op=OP.subtract)
                    s2t = scr.tile([128, 512], f32, tag="ow", name="s2t")
                    nc.vector.tensor_tensor(out=s2t[:], in0=s1t[:], in1=b2t[:],
                                            op=OP.subtract)
                    m1t = scr.tile([128, 512], f32, tag="ow", name="m1t")
                    nc.vector.tensor_tensor(out=m1t[:], in0=b_g2[:],
                                            in1=slts[tt][:], op=OP.mult)
                    o1t = scr.tile([128, 512], f32, tag="ow", name="o1t")
                    nc.vector.tensor_tensor(out=o1t[:], in0=s2t[:], in1=m1t[:],
                                            op=OP.subtract)
                    m3t = scr.tile([128, 512], f32, tag="ow", name="m3t")
                    nc.vector.tensor_tensor(out=m3t[:], in0=b_g3[:],
                                            in1=lsts[tt][:], op=OP.mult)
                    ot = smp.tile([128, 512], f32, tag="ot", name="ot")
                    nc.vector.tensor_tensor(out=ot[:], in0=o1t[:], in1=m3t[:],
                                            op=OP.subtract)
                    for cc in range(4):
                        pt2 = pst_()
                        nc.tensor.transpose(
                            out=pt2[0:128, 0:128],
                            in_=ot[:, cc * 128 : (cc + 1) * 128],
                            identity=id_s[:],
                        )
                        ob = smp.tile([128, 128], f32, tag="ob", name="ob")
                        nc.vector.tensor_copy(out=ob[:], in_=pt2[0:128, 0:128])
                        r0 = t * NB + cc * 128
                        nc.sync.dma_start(
                            out=out[tt, r0 : r0 + 128, :], in_=ob[:]
                        )

                xT_cur = xT_next

    nc.compile()
    _BUILT["nc"] = nc
    return nc


def _w0np():
    if W0_BF16:
        import ml_dtypes

        return ml_dtypes.bfloat16
    return np.float32


def _prep_core(x_c, d_c):
    """Build idx16 (128, NT*NSUB*NIDX/16) int16 and qv (128, NT*128) f32."""
    xv = np.asarray(x_c, dtype=np.int64)  # (BC, F)
    idx16 = np.zeros((16, NT * NSUB * NIDX // 16), dtype=np.int16)
    qv = np.zeros((P, NT * P), dtype=np.float32)
    for t in range(NT):
        xt = xv[t * NB : (t + 1) * NB]  # (512, 32)
        vj = np.empty((P, P), dtype=np.int64)  # [j, p], j = cc*32 + f
        for cc in range(4):
            blk = xt[cc * 128 : (cc + 1) * 128]  # (128 p, 32 f)
            vj[cc * 32 : (cc + 1) * 32, :] = blk.T
        qv[:, t * P : (t + 1) * P] = (vj & 3).T.astype(np.float32)
        hi = (vj >> 2).astype(np.int16)  # [j, p]
        for s2 in range(NSUB):
            sub = hi[s2 * SLOT : (s2 + 1) * SLOT, :]  # [c, p]; i = c*128 + p
            flat = sub.reshape(-1)
            wrapped = flat.reshape(NIDX // 16, 16).T  # idx16[ch, col] = flat[col*16+ch]
            c0 = t * (NSUB * NIDX // 16) + s2 * (NIDX // 16)
            idx16[:, c0 : c0 + NIDX // 16] = wrapped
    idx_full = np.tile(idx16, (8, 1))
    dfm = np.asarray(d_c, dtype=np.float32).reshape(1, BC)
    return idx_full, qv, dfm


TRACE = False
LAST_RESULT = None


def kernel(**inputs):
    from concourse.bass_utils import run_bass_kernel_spmd

    nc = build()

    f32 = np.float32
    emb = np.asarray(inputs["emb"], dtype=f32)
    emb4 = np.ascontiguousarray(emb.reshape(V4, E4))
    x = np.asarray(inputs["x"], dtype=np.int64)
    d = np.asarray(inputs["d"], dtype=np.int64)

    Tb0 = np.asarray(inputs["Tb0"], f32)
    Tlb0 = np.asarray(inputs["Tlb0"], f32)
    Tb1 = np.asarray(inputs["Tb1"], f32)
    Tlb1 = np.asarray(inputs["Tlb1"], f32)
    Tk1 = np.asarray(inputs["Tk1"], f32)
    TA1 = np.asarray(inputs["TA1"], f32)
    TB1 = np.asarray(inputs["TB1"], f32)
    TB0 = np.asarray(inputs["TB0"], f32)
    TA0 = np.asarray(inputs["TA0"], f32)
    DA0 = np.asarray(inputs["DA0"], f32)
    DB0 = np.asarray(inputs["DB0"], f32)
    Dlb0 = np.asarray(inputs["Dlb0"], f32)
    h0W = np.asarray(inputs["h0W"], f32)
    h1W = np.asarray(inputs["h1W"], f32)
    h0b = np.asarray(inputs["h0b"], f32)
    h1b = np.asarray(inputs["h1b"], f32)

    pk1 = []
    for tt in range(2):
        pkt = np.zeros((128, 32), f32)
        pkt[:, 0:16] = TA1[tt]
        pkt[:, 16] = Tk1[:, 0]
        pk1.append(pkt)

    def c128(v):
        return np.full((128, 1), v, f32)

    def kmajor(W, k):
        """Rearrange (k*128, N) so a row-major DMA into a (128, k, N) SBUF
        tile yields tile[p, j] = W[j*128 + p]."""
        W = np.asarray(W, f32)
        n = W.shape[1] if W.ndim > 1 else 1
        return np.ascontiguousarray(
            W.reshape(k, 128, n).transpose(1, 0, 2).reshape(k * 128, n)
        )

    # one-hot row-broadcast selectors
    E32 = np.zeros((32, 128), f32)
    E32[16, :] = 1.0
    G8sel = np.zeros((8, 8 * 128), f32)
    for r in range(8):
        G8sel[r, r * 128 : (r + 1) * 128] = 1.0

    diota = (np.arange(128) // 16).astype(f32).reshape(128, 1)

    shared = {
        "emb4": emb4,
        "sW0": np.asarray(inputs["sW0"], _w0np()),
        "gW0": np.asarray(inputs["gW0"], _w0np()),
        "sW1": np.asarray(inputs["sW1"], f32),
        "gW1": np.asarray(inputs["gW1"], f32),
        "sb0": kmajor(np.asarray(inputs["sb0"], f32).reshape(1024, 1), 8),
        "gb0": kmajor(np.asarray(inputs["gb0"], f32).reshape(1024, 1), 8),
        "sb1": kmajor(np.asarray(inputs["sb1"], f32).reshape(512, 1), 4),
        "gb1": kmajor(np.asarray(inputs["gb1"], f32).reshape(512, 1), 4),
        "Dk0": kmajor(inputs["Dk0"], 4),
        "Db0": kmajor(np.asarray(inputs["Db0"], f32).reshape(256, 1), 2),
        "DA0p": kmajor(DA0.transpose(1, 0, 2).reshape(512, 128), 4),
        "DB0p": np.ascontiguousarray(DB0.reshape(128, 256)),
        "Dlb0p": np.ascontiguousarray(Dlb0),
        "Tk0": kmajor(inputs["Tk0"], 2),
        "TA0_0": kmajor(TA0[0], 2),
        "TA0_1": kmajor(TA0[1], 2),
        "TB0_0": np.ascontiguousarray(TB0[0]),
        "TB0_1": np.ascontiguousarray(TB0[1]),
        "bias_d0": (Tb0 + Tlb0[0]).reshape(128, 1),
        "bias_d1": (Tb0 + Tlb0[1]).reshape(128, 1),
        "bias_s": Tb0.reshape(128, 1),
        "bias_l0": Tlb0[0].reshape(128, 1),
        "bias_l1": Tlb0[1].reshape(128, 1),
        "pk1_0": pk1[0],
        "pk1_1": pk1[1],
        "TA1_0": np.ascontiguousarray(TA1[0]),
        "TA1_1": np.ascontiguousarray(TA1[1]),
        "Tk1t": Tk1,
        "TB1_0": np.ascontiguousarray(TB1[0]),
        "TB1_1": np.ascontiguousarray(TB1[1]),
        "c_d0": c128(Tb1[0] + Tlb1[0, 0]),
        "c_d1": c128(Tb1[0] + Tlb1[1, 0]),
        "c_s": c128(Tb1[0]),
        "c_l0": c128(Tlb1[0, 0]),
        "c_l1": c128(Tlb1[1, 0]),
        "HWp": kmajor(np.concatenate([h0W, h1W], axis=1), 4),
        "hb": np.concatenate([h0b, h1b]).reshape(8, 1),
        "onesr": np.ones((1, 128), f32),
        "diota": diota,
        "iota8": np.arange(8, dtype=f32).reshape(8, 1),
        "ident": np.eye(128, dtype=f32),
        "E32": E32,
        "G8sel": G8sel,
    }

    in_maps = []
    for c in range(NCORES):
        idx_full, qv, dfm = _prep_core(
            x[c * BC : (c + 1) * BC], d[c * BC : (c + 1) * BC]
        )
        m = dict(shared)
        m["idxd"] = idx_full
        m["qvd"] = qv
        m["dfm"] = dfm
        in_maps.append(m)

    res = run_bass_kernel_spmd(nc, in_maps, list(range(NCORES)), trace=TRACE)
    global LAST_RESULT
    LAST_RESULT = res
    outs = [res.results[c]["out"] for c in range(NCORES)]
    return np.concatenate(outs, axis=1)  # (2, B, 128)



# revision 6
# speedup vs baseline: 2.0028x; 2.0028x over previous
"""Trainium2 Bass kernel for nn_AutoIFS_89378269430151 (moe_routing).

Data-parallel over batch across 8 NeuronCores; all params replicated and
RESIDENT in SBUF (loaded once in the prologue). Feature-major layout on
chip (features on partitions, batch on free dim). Embedding gather via
SWDGE dma_gather on a (25000, 256) fp8-e3m4 (x16 pre-scaled) reshaped view
of the table with int16 indices (idx = x>>2); quarter (x&3) selected on
DVE with host-precomputed masks. All 8 sub-gathers of a tile are issued
upfront so the 4 SWDGE queues' Q7 core pairs generate descriptors
concurrently; the selects lag behind the gathers so they never head-block
the DVE queue. Tower matmuls in bf16 (x16 scale undone in the l0
activation epilogue). Output returned bf16 and upcast on host.
"""

import os

os.environ.setdefault("JAX_PLATFORMS", "axon")

import numpy as np

# ---- problem constants (hardcoded; must match reference.py) ----
B, F, L = 16384, 32, 64
FEAT, DOM, R = 100000, 8, 16
D_IN = F * L  # 2048
NCORES = 8
BC = B // NCORES  # 2048 samples per core
NB = 512  # batch tile (free dim)
NT = BC // NB  # 4 batch tiles per core
P = 128

V4 = FEAT // 4  # 25000 rows in reshaped table
E4 = 4 * L  # 256 elements per reshaped row (fp8 -> 256B)
NSUB = 8  # sub-gathers per batch tile
NIDX = NB * F // NSUB  # 2048 indices per sub-gather
SLOT = NIDX // P  # 16 slots per sub-gather
ESCALE = 16.0  # table pre-scale (undone in l0 activation)

_BUILT = {}


def build():
    if "nc" in _BUILT:
        return _BUILT["nc"]
    import concourse.bass as bass
    from concourse import bacc
    import concourse.mybir as mybir
    import concourse.tile as tile

    dt = mybir.dt
    AF = mybir.ActivationFunctionType
    OP = mybir.AluOpType
    f32, f32r, i16, bf16 = dt.float32, dt.float32r, dt.int16, dt.bfloat16
    f8 = dt.float8e3

    nc = bacc.Bacc(None, target_bir_lowering=False, num_swdge_queues=4)

    def din(name, shape, dtype=f32):
        return nc.dram_tensor(name, shape, dtype, kind="ExternalInput")

    # ---- DRAM inputs (per core) ----
    emb4 = din("emb4", [V4, E4], f8)  # reshaped fp8 embedding table (x16)
    idxd = din("idxd", [P, NT * NSUB * NIDX // 16], i16)  # wrapped gather idx
    mqd = din("mqd", [P, NT * 3 * P], dt.uint8)  # quarter masks q=1,2,3
    dfm = din("dfm", [1, BC], f32r)  # domain ids (f32 bits, feature-major row)

    sW0 = din("sW0", [D_IN, 1024], bf16)
    gW0 = din("gW0", [D_IN, 1024], bf16)
    sW1 = din("sW1", [1024, 512], bf16)
    gW1 = din("gW1", [1024, 512], bf16)
    sb0 = din("sb0", [1024, 1])
    gb0 = din("gb0", [1024, 1])
    sb1 = din("sb1", [512, 1])
    gb1 = din("gb1", [512, 1])

    Dk0 = din("Dk0", [512, 256], bf16)
    Db0 = din("Db0", [256, 1])
    DA0p = din("DA0p", [512, 128], bf16)  # [i, dom*16+r]
    DB0p = din("DB0p", [128, 256], bf16)  # [dom*16+r, o]
    Dlb0p = din("Dlb0p", [8, 256], bf16)  # [dom, o]

    Tk0 = din("Tk0", [256, 128], bf16)
    TA0_0 = din("TA0_0", [256, 16], bf16)
    TA0_1 = din("TA0_1", [256, 16], bf16)
    TB0_0 = din("TB0_0", [16, 128], bf16)
    TB0_1 = din("TB0_1", [16, 128], bf16)
    bias_d0 = din("bias_d0", [128, 1])  # Tb0 + Tlb0[0]
    bias_d1 = din("bias_d1", [128, 1])  # Tb0 + Tlb0[1]
    bias_s = din("bias_s", [128, 1])  # Tb0
    bias_l0 = din("bias_l0", [128, 1])  # Tlb0[0]
    bias_l1 = din("bias_l1", [128, 1])  # Tlb0[1]

    pk1_0 = din("pk1_0", [128, 32], bf16)  # cols 0:16 TA1[0], col 16 Tk1
    pk1_1 = din("pk1_1", [128, 32], bf16)
    TA1_0 = din("TA1_0", [128, 16], bf16)
    TA1_1 = din("TA1_1", [128, 16], bf16)
    # rank-1 row-broadcast matrices for the i=1 epilogue
    FD1_0 = din("FD1_0", [32, 128], bf16)  # rows 0:16 = TB1[0] col, row 16 = 1
    FD1_1 = din("FD1_1", [32, 128], bf16)
    LB1_0 = din("LB1_0", [16, 128], bf16)  # = TB1[0] broadcast along cols
    LB1_1 = din("LB1_1", [16, 128], bf16)
    K1b = din("K1b", [128, 128], bf16)  # = Tk1 broadcast along cols
    # per-partition scalar constants (value replicated on 128 partitions)
    c_d0 = din("c_d0", [128, 1])  # Tb1 + Tlb1[0]
    c_d1 = din("c_d1", [128, 1])  # Tb1 + Tlb1[1]
    c_s = din("c_s", [128, 1])  # Tb1
    c_l0 = din("c_l0", [128, 1])  # Tlb1[0]
    c_l1 = din("c_l1", [128, 1])  # Tlb1[1]

    HWp = din("HWp", [512, 8], bf16)  # [h0W | h1W]
    hb = din("hb", [8, 1])

    onesr = din("onesr", [1, 128], f32r)
    diota = din("diota", [128, 1])  # p // 16
    iota8 = din("iota8", [8, 1])
    identb = din("identb", [128, 128], bf16)
    G8sel = din("G8sel", [8, 8 * 128], bf16)  # row-broadcast for gates rows 0..7

    out = nc.dram_tensor("out", [2, BC, 128], bf16, kind="ExternalOutput")

    with tile.TileContext(nc) as tc:
        with (
            tc.tile_pool(name="const", bufs=1) as cp,
            tc.tile_pool(name="gpool", bufs=6) as gp,
            tc.tile_pool(name="selpool", bufs=2) as sp,
            tc.tile_pool(name="xt", bufs=2) as xp,
            tc.tile_pool(name="hpool", bufs=1) as hp,
            tc.tile_pool(name="task", bufs=1) as tkp,
            tc.tile_pool(name="small", bufs=2) as smp,
            tc.tile_pool(name="scratch", bufs=4) as scr,
            tc.tile_pool(name="bounce", bufs=2) as bnc,
            tc.tile_pool(name="ps", bufs=6, space="PSUM") as psp,
            tc.tile_pool(name="pt", bufs=2, space="PSUM") as ptp,
        ):
            def pst_():
                return psp.tile([128, 512], dt.float32, space="PSUM", tag="ps",
                                name="pstile")

            # ---- resident tensors ----
            def load_const(t, shape, dtype=f32):
                s = cp.tile(shape, dtype, tag=t.name)
                nc.sync.dma_start(out=s[:], in_=t[:])
                return s

            # big resident weights (gate tower first: it runs first)
            gW0_s = load_const(gW0, [128, 16, 1024], bf16)
            gW1_s = load_const(gW1, [128, 8, 512], bf16)
            sW0_s = load_const(sW0, [128, 16, 1024], bf16)
            sW1_s = load_const(sW1, [128, 8, 512], bf16)

            sb0_s = load_const(sb0, [128, 8, 1])
            gb0_s = load_const(gb0, [128, 8, 1])
            sb1_s = load_const(sb1, [128, 4, 1])
            gb1_s = load_const(gb1, [128, 4, 1])
            Dk0_s = load_const(Dk0, [128, 4, 256], bf16)
            Db0_s = load_const(Db0, [128, 2, 1])
            DA0p_s = load_const(DA0p, [128, 4, 128], bf16)
            DB0p_s = load_const(DB0p, [128, 256], bf16)
            Dlb0p_s = load_const(Dlb0p, [8, 256], bf16)
            Tk0_s = load_const(Tk0, [128, 2, 128], bf16)
            TA0_0s = load_const(TA0_0, [128, 2, 16], bf16)
            TA0_1s = load_const(TA0_1, [128, 2, 16], bf16)
            TB0_0s = load_const(TB0_0, [16, 128], bf16)
            TB0_1s = load_const(TB0_1, [16, 128], bf16)
            bd0_s = load_const(bias_d0, [128, 1])
            bd1_s = load_const(bias_d1, [128, 1])
            bs_s = load_const(bias_s, [128, 1])
            bl0_s = load_const(bias_l0, [128, 1])
            bl1_s = load_const(bias_l1, [128, 1])
            pk1_0s = load_const(pk1_0, [128, 32], bf16)
            pk1_1s = load_const(pk1_1, [128, 32], bf16)
            TA1_0s = load_const(TA1_0, [128, 16], bf16)
            TA1_1s = load_const(TA1_1, [128, 16], bf16)
            FD1_s = [load_const(FD1_0, [32, 128], bf16),
                     load_const(FD1_1, [32, 128], bf16)]
            LB1_s = [load_const(LB1_0, [16, 128], bf16),
                     load_const(LB1_1, [16, 128], bf16)]
            K1b_s = load_const(K1b, [128, 128], bf16)
            cd_s = [load_const(c_d0, [128, 1]), load_const(c_d1, [128, 1])]
            cs_s = load_const(c_s, [128, 1])
            cl_s = [load_const(c_l0, [128, 1]), load_const(c_l1, [128, 1])]
            HW_s = load_const(HWp, [128, 4, 8], bf16)
            hb_s = load_const(hb, [8, 1])
            ones_s = load_const(onesr, [1, 128], f32r)
            diota_s = load_const(diota, [128, 1])
            iota8_s = load_const(iota8, [8, 1])
            idb_s = load_const(identb, [128, 128], bf16)
            G8_s = load_const(G8sel, [8, 8, 128], bf16)

            ICOLS = NSUB * NIDX // 16  # idx cols per tile (1024)

            def load_tile_inputs(t):
                idx_t = smp.tile([P, ICOLS], i16, tag="idx", name="idx_t")
                nc.scalar.dma_start(
                    out=idx_t[:], in_=idxd[:, t * ICOLS : (t + 1) * ICOLS]
                )
                mq_t = smp.tile([P, 3, P], dt.uint8, tag="mq", name="mq_t")
                nc.scalar.dma_start(
                    out=mq_t[:], in_=mqd[:, t * 3 * P : (t + 1) * 3 * P]
                )
                dfm_t = smp.tile([1, NB], f32r, tag="dfm", name="dfm_t")
                nc.scalar.dma_start(
                    out=dfm_t[:], in_=dfm[:, t * NB : (t + 1) * NB]
                )
                return idx_t, mq_t, dfm_t

            def issue_gathers(idx_t):
                gs = []
                for j in range(NSUB):
                    g = gp.tile([P, SLOT, E4], f8, tag="g", name="g")
                    c0 = j * (NIDX // 16)
                    nc.gpsimd.dma_gather(
                        out_ap=g[:],
                        in_ap=emb4[:],
                        idxs_ap=idx_t[:, c0 : c0 + NIDX // 16],
                        num_idxs=NIDX,
                        num_idxs_reg=NIDX,
                        elem_size=E4,
                        single_packet=False,
                        queue_num=j % 4,
                    )
                    gs.append(g)
                return gs

            def make_selector(gs, mq_t, xT_dst):
                state = {"j": 0}

                def select(n=1):
                    for _ in range(n):
                        j = state["j"]
                        if j >= NSUB:
                            return
                        state["j"] += 1
                        g = gs[j]
                        sel = sp.tile([P, SLOT, L], bf16, tag="sel", name="sel")
                        nc.vector.tensor_copy(out=sel[:], in_=g[:, :, 0:L])
                        for q in range(1, 4):
                            nc.vector.copy_predicated(
                                out=sel[:],
                                mask=mq_t[:, q - 1, j * SLOT : (j + 1) * SLOT,
                                          None].to_broadcast([P, SLOT, L]),
                                data=g[:, :, q * L : (q + 1) * L],
                            )
                        cc = (j * SLOT) // 32
                        ff0 = ((j * SLOT) % 32) // 2
                        for half in range(2):
                            pt = ptp.tile([128, 4, 128], bf16, space="PSUM",
                                          tag="pt", name="pt")
                            for f2 in range(4):
                                nc.tensor.transpose(
                                    out=pt[:, f2, :],
                                    in_=sel[:, 8 * half + 2 * f2
                                            : 8 * half + 2 * f2 + 2, :],
                                    identity=idb_s[:],
                                )
                            nc.vector.tensor_copy(
                                out=xT_dst[:, ff0 + 4 * half : ff0 + 4 * half + 4,
                                           cc * 128 : (cc + 1) * 128],
                                in_=pt[:],
                            )

                return select

            def tower_l0(W_s, bias_tile, xT, pump, scale=1.0):
                h0 = hp.tile([128, 8, 512], bf16, tag="h0", name="h0")
                for half in range(2):
                    pst = [pst_() for _ in range(4)]
                    for k in range(16):
                        for m in range(4):
                            nc.tensor.matmul(
                                out=pst[m][:],
                                lhsT=W_s[:, k,
                                         half * 512 + m * 128
                                         : half * 512 + (m + 1) * 128],
                                rhs=xT[:, k, :],
                                start=(k == 0),
                                stop=(k == 15),
                            )
                        if k % 4 == 3:
                            pump()
                    for m in range(4):
                        mm = half * 4 + m
                        nc.scalar.activation(
                            out=h0[:, mm, :], in_=pst[m][:], func=AF.Relu,
                            bias=bias_tile[:, mm, :], scale=scale,
                        )
                return h0

            def tower_l1(h0, W1_s, b1s, pump):
                h1 = hp.tile([128, 4, 512], bf16, tag="h1", name="h1")
                pst = [pst_() for _ in range(4)]
                for k in range(8):
                    for m in range(4):
                        nc.tensor.matmul(
                            out=pst[m][:],
                            lhsT=W1_s[:, k, m * 128 : (m + 1) * 128],
                            rhs=h0[:, k, :],
                            start=(k == 0),
                            stop=(k == 7),
                        )
                    if k % 4 == 3:
                        pump()
                for m in range(4):
                    nc.scalar.activation(
                        out=h1[:, m, :], in_=pst[m][:], func=AF.Relu,
                        bias=b1s[:, m, :],
                    )
                return h1

            noop = lambda n=1: None

            # -------- prologue: tile 0 gathers + select --------
            idx_c, mq_c, dfm_c = load_tile_inputs(0)
            xT_cur = xp.tile([128, 16, 512], bf16, tag="xT", name="xT0")
            gs0 = issue_gathers(idx_c)
            sel0 = make_selector(gs0, mq_c, xT_cur)
            sel0(NSUB)

            for t in range(NT):
                dfm_t = dfm_c
                if t + 1 < NT:
                    idx_n, mq_n, dfm_c = load_tile_inputs(t + 1)
                    gs = issue_gathers(idx_n)
                    xT_next = xp.tile([128, 16, 512], bf16, tag="xT", name="xTn")
                    pump = make_selector(gs, mq_n, xT_next)
                else:
                    xT_next = None
                    pump = noop
                xT = xT_cur

                # ================= gate tower + gates =================
                h0g = tower_l0(gW0_s, gb0_s, xT, noop, scale=1.0 / ESCALE)
                hyper = tower_l1(h0g, gW1_s, gb1_s, pump)
                psg = pst_()
                for k in range(4):
                    nc.tensor.matmul(
                        out=psg[0:8, :], lhsT=HW_s[:, k, :], rhs=hyper[:, k, :],
                        start=(k == 0), stop=(k == 3),
                    )
                gates = tkp.tile([8, 512], bf16, tag="gates", name="gates")
                nc.scalar.activation(
                    out=gates[:], in_=psg[0:8, :], func=AF.Sigmoid, bias=hb_s[:]
                )

                # ================= share tower =================
                h0s = tower_l0(sW0_s, sb0_s, xT, pump, scale=1.0 / ESCALE)
                h1s = tower_l1(h0s, sW1_s, sb1_s, pump)
                pump(NSUB)  # flush any remaining next-tile selects

                # ================= domain layer =================
                psd = pst_()
                nc.tensor.matmul(
                    out=psd[:], lhsT=ones_s[:], rhs=dfm_t[:],
                    start=True, stop=True,
                )
                mbc = tkp.tile([128, 512], bf16, tag="mbc", name="mbc")
                nc.vector.tensor_scalar(
                    out=mbc[:], in0=psd[:], scalar1=diota_s[:], scalar2=None,
                    op0=OP.is_equal,
                )
                mask8 = tkp.tile([8, 512], bf16, tag="mask8", name="mask8")
                nc.vector.tensor_scalar(
                    out=mask8[:], in0=psd[0:8, :], scalar1=iota8_s[:], scalar2=None,
                    op0=OP.is_equal,
                )
                psR = pst_()
                for k in range(4):
                    nc.tensor.matmul(
                        out=psR[:], lhsT=DA0p_s[:, k, :], rhs=h1s[:, k, :],
                        start=(k == 0), stop=(k == 3),
                    )
                rmask = tkp.tile([128, 512], bf16, tag="rmask", name="rmask")
                nc.vector.tensor_tensor(
                    out=rmask[:], in0=psR[:], in1=mbc[:], op=OP.mult
                )
                share = tkp.tile([128, 2, 512], bf16, tag="share", name="share")
                lora = tkp.tile([128, 2, 512], bf16, tag="lora", name="lora")
                ddnn = tkp.tile([128, 2, 512], bf16, tag="ddnn", name="ddnn")
                for m in range(2):
                    pss = pst_()
                    for k in range(4):
                        nc.tensor.matmul(
                            out=pss[:],
                            lhsT=Dk0_s[:, k, m * 128 : (m + 1) * 128],
                            rhs=h1s[:, k, :],
                            start=(k == 0), stop=(k == 3),
                        )
                    psl = pst_()
                    nc.tensor.matmul(
                        out=psl[:], lhsT=DB0p_s[:, m * 128 : (m + 1) * 128],
                        rhs=rmask[:], start=True, stop=False,
                    )
                    nc.tensor.matmul(
                        out=psl[:], lhsT=Dlb0p_s[:, m * 128 : (m + 1) * 128],
                        rhs=mask8[:], start=False, stop=True,
                    )
                    nc.scalar.activation(
                        out=share[:, m, :], in_=pss[:], func=AF.Identity,
                        bias=Db0_s[:, m, :],
                    )
                    nc.vector.tensor_copy(out=lora[:, m, :], in_=psl[:])
                    tmp = scr.tile([128, 512], f32, tag="ow", name="tmp")
                    nc.vector.tensor_tensor(
                        out=tmp[:], in0=share[:, m, :],
                        in1=lora[:, m, :], op=OP.add
                    )
                    nc.scalar.activation(
                        out=ddnn[:, m, :], in_=tmp[:], func=AF.Relu,
                    )

                # ================= task layer i=0 =================
                def aprod(TA_s, rhs3):
                    pa = pst_()
                    nc.tensor.matmul(out=pa[0:16, :], lhsT=TA_s[:, 0, :],
                                     rhs=rhs3[:, 0, :], start=True, stop=False)
                    nc.tensor.matmul(out=pa[0:16, :], lhsT=TA_s[:, 1, :],
                                     rhs=rhs3[:, 1, :], start=False, stop=True)
                    ab = bnc.tile([16, 512], bf16, tag="Abuf", name="ab")
                    nc.vector.tensor_copy(out=ab[:], in_=pa[0:16, :])
                    return ab

                def mm_k2(pstile, lhs_tile, rhs3, last):
                    nc.tensor.matmul(out=pstile[:], lhsT=lhs_tile[:, 0, :],
                                     rhs=rhs3[:, 0, :], start=True, stop=False)
                    nc.tensor.matmul(out=pstile[:], lhsT=lhs_tile[:, 1, :],
                                     rhs=rhs3[:, 1, :], start=False, stop=last)

                ab_d0 = aprod(TA0_0s, ddnn)
                ps_d0 = pst_()
                mm_k2(ps_d0, Tk0_s, ddnn, last=False)
                nc.tensor.matmul(out=ps_d0[:], lhsT=TB0_0s[:], rhs=ab_d0[:],
                                 start=False, stop=True)
                ab_d1 = aprod(TA0_1s, ddnn)
                ps_d1 = pst_()
                mm_k2(ps_d1, Tk0_s, ddnn, last=False)
                nc.tensor.matmul(out=ps_d1[:], lhsT=TB0_1s[:], rhs=ab_d1[:],
                                 start=False, stop=True)
                ab_s0 = aprod(TA0_0s, share)
                ps_sl0 = pst_()
                nc.tensor.matmul(out=ps_sl0[:], lhsT=TB0_0s[:], rhs=ab_s0[:],
                                 start=True, stop=True)
                ab_s1 = aprod(TA0_1s, share)
                ps_sl1 = pst_()
                nc.tensor.matmul(out=ps_sl1[:], lhsT=TB0_1s[:], rhs=ab_s1[:],
                                 start=True, stop=True)
                ab_l0 = aprod(TA0_0s, lora)
                ps_ll0 = pst_()
                nc.tensor.matmul(out=ps_ll0[:], lhsT=TB0_0s[:], rhs=ab_l0[:],
                                 start=True, stop=True)
                ab_l1 = aprod(TA0_1s, lora)
                ps_ll1 = pst_()
                nc.tensor.matmul(out=ps_ll1[:], lhsT=TB0_1s[:], rhs=ab_l1[:],
                                 start=True, stop=True)
                ps_S = pst_()
                mm_k2(ps_S, Tk0_s, share, last=True)
                ps_L = pst_()
                mm_k2(ps_L, Tk0_s, lora, last=True)

                def epi_relu(pstile, bias, tag="tk"):
                    o = tkp.tile([128, 512], bf16, tag=tag, name="epi")
                    nc.scalar.activation(out=o[:], in_=pstile[:], func=AF.Relu,
                                         bias=bias[:])
                    return o

                def epi_add(pstile, bias, tag="tk"):
                    o = tkp.tile([128, 512], bf16, tag=tag, name="epi")
                    nc.scalar.activation(out=o[:], in_=pstile[:],
                                         func=AF.Identity, bias=bias[:])
                    return o

                t_dnn0 = epi_relu(ps_d0, bd0_s, tag="tdnn0")
                t_dnn1 = epi_add(ps_d1, bd1_s, tag="tdnn1")
                t_sh0 = epi_relu(ps_S, bs_s, tag="tsh0")
                t_sh1 = epi_add(ps_S, bs_s, tag="tsh1")
                t_ls0 = epi_relu(ps_L, bs_s, tag="tls0")
                t_ls1 = epi_add(ps_L, bs_s, tag="tls1")
                t_lo0 = epi_relu(ps_ll0, bl0_s, tag="tlo0")
                t_lo1 = epi_add(ps_ll1, bl1_s, tag="tlo1")
                t_sl0 = epi_relu(ps_sl0, bl0_s, tag="tsl0")
                t_sl1 = epi_add(ps_sl1, bl1_s, tag="tsl1")

                # ================= task layer i=1 + output =================
                pk1s = [pk1_0s, pk1_1s]
                TA1s = [TA1_0s, TA1_1s]
                tdnns = [t_dnn0, t_dnn1]
                tshs = [t_sh0, t_sh1]
                tlos = [t_lo0, t_lo1]
                slts = [t_sl0, t_sl1]
                lsts = [t_ls0, t_ls1]

                for tt in range(2):
                    # F block: rows 0:16 = dA1, row 16 = ds
                    pf = pst_()
                    nc.tensor.matmul(out=pf[0:32, :], lhsT=pk1s[tt][:],
                                     rhs=tdnns[tt][:], start=True, stop=True)
                    fb = bnc.tile([32, 512], bf16, tag="Fbuf", name="fb")
                    nc.vector.tensor_copy(out=fb[:], in_=pf[0:32, :])
                    # lA1
                    pl = pst_()
                    nc.tensor.matmul(out=pl[0:16, :], lhsT=TA1s[tt][:],
                                     rhs=tlos[tt][:], start=True, stop=True)
                    lb = bnc.tile([16, 512], bf16, tag="Abuf", name="lb")
                    nc.vector.tensor_copy(out=lb[:], in_=pl[0:16, :])

                    # rank-1 broadcasts: b_w = (ds + dl) rows, b_ss, b_ll
                    b_ss = pst_()
                    nc.tensor.matmul(out=b_ss[:], lhsT=K1b_s[:], rhs=tshs[tt][:],
                                     start=True, stop=True)
                    b_g0 = pst_()
                    nc.tensor.matmul(out=b_g0[:], lhsT=G8_s[:, 4 * tt, :],
                                     rhs=gates[:], start=True, stop=True)
                    a1t = scr.tile([128, 512], bf16, tag="ow", name="a1t")
                    nc.scalar.activation(out=a1t[:], in_=b_ss[:],
                                         func=AF.Identity, bias=cs_s[:])
                    a2t = scr.tile([128, 512], bf16, tag="ow", name="a2t")
                    nc.vector.tensor_tensor(out=a2t[:], in0=a1t[:], in1=b_g0[:],
                                            op=OP.mult)

                    b_ll = pst_()
                    nc.tensor.matmul(out=b_ll[:], lhsT=LB1_s[tt][:], rhs=lb[:],
                                     start=True, stop=True)
                    b_g1 = pst_()
                    nc.tensor.matmul(out=b_g1[:], lhsT=G8_s[:, 4 * tt + 1, :],
                                     rhs=gates[:], start=True, stop=True)
                    b1t = scr.tile([128, 512], bf16, tag="ow", name="b1t")
                    nc.scalar.activation(out=b1t[:], in_=b_ll[:],
                                         func=AF.Identity, bias=cl_s[tt][:])
                    b2t = scr.tile([128, 512], bf16, tag="ow", name="b2t")
                    nc.vector.tensor_tensor(out=b2t[:], in0=b1t[:], in1=b_g1[:],
                                            op=OP.mult)

                    b_w = pst_()
                    nc.tensor.matmul(out=b_w[:], lhsT=FD1_s[tt][:], rhs=fb[:],
                                     start=True, stop=True)
                    w1t = scr.tile([128, 512], bf16, tag="ow", name="w1t")
                    nc.scalar.activation(out=w1t[:], in_=b_w[:],
                                         func=AF.Identity, bias=cd_s[tt][:])
                    s1t = scr.tile([128, 512], f32, tag="ow", name="s1t")
                    nc.vector.tensor_tensor(out=s1t[:], in0=w1t[:], in1=a2t[:],
                                            op=OP.subtract)
                    s2t = scr.tile([128, 512], f32, tag="ow", name="s2t")
                    nc.vector.tensor_tensor(out=s2t[:], in0=s1t[:], in1=b2t[:],
                                            op=OP.subtract)
                    b_g2 = pst_()
                    nc.tensor.matmul(out=b_g2[:], lhsT=G8_s[:, 4 * tt + 2, :],
                                     rhs=gates[:], start=True, stop=True)
                    m1t = scr.tile([128, 512], f32, tag="ow", name="m1t")
                    nc.vector.tensor_tensor(out=m1t[:], in0=b_g2[:],
                                            in1=slts[tt][:], op=OP.mult)
                    o1t = scr.tile([128, 512], f32, tag="ow", name="o1t")
                    nc.vector.tensor_tensor(out=o1t[:], in0=s2t[:], in1=m1t[:],
                                            op=OP.subtract)
                    b_g3 = pst_()
                    nc.tensor.matmul(out=b_g3[:], lhsT=G8_s[:, 4 * tt + 3, :],
                                     rhs=gates[:], start=True, stop=True)
                    m3t = scr.tile([128, 512], f32, tag="ow", name="m3t")
                    nc.vector.tensor_tensor(out=m3t[:], in0=b_g3[:],
                                            in1=lsts[tt][:], op=OP.mult)
                    ot = smp.tile([128, 512], bf16, tag="ot", name="ot")
                    nc.vector.tensor_tensor(out=ot[:], in0=o1t[:], in1=m3t[:],
                                            op=OP.subtract)
                    for cc in range(4):
                        pt2 = ptp.tile([128, 128], bf16, space="PSUM",
                                       tag="pt", name="pt2")
                        nc.tensor.transpose(
                            out=pt2[0:128, 0:128],
                            in_=ot[:, cc * 128 : (cc + 1) * 128],
                            identity=idb_s[:],
                        )
                        ob = smp.tile([128, 128], bf16, tag="ob", name="ob")
                        nc.vector.tensor_copy(out=ob[:], in_=pt2[0:128, 0:128])
                        r0 = t * NB + cc * 128
                        nc.sync.dma_start(
                            out=out[tt, r0 : r0 + 128, :], in_=ob[:]
                        )

                xT_cur = xT_next

    nc.compile()
    _BUILT["nc"] = nc
    return nc


def _prep_core(x_c, d_c):
    """Build idx16 (128, NT*NSUB*NIDX/16) int16 and quarter masks
    (128, NT*3*128) uint8."""
    xv = np.asarray(x_c, dtype=np.int64)  # (BC, F)
    idx16 = np.zeros((16, NT * NSUB * NIDX // 16), dtype=np.int16)
    mq = np.zeros((P, NT * 3 * P), dtype=np.uint8)
    for t in range(NT):
        xt = xv[t * NB : (t + 1) * NB]  # (512, 32)
        vj = np.empty((P, P), dtype=np.int64)  # [j, p], j = cc*32 + f
        for cc in range(4):
            blk = xt[cc * 128 : (cc + 1) * 128]  # (128 p, 32 f)
            vj[cc * 32 : (cc + 1) * 32, :] = blk.T
        qv = (vj & 3).T  # [p, j]
        for q in (1, 2, 3):
            mq[:, (t * 3 + q - 1) * P : (t * 3 + q) * P] = (qv == q)
        hi = (vj >> 2).astype(np.int16)  # [j, p]
        for s2 in range(NSUB):
            sub = hi[s2 * SLOT : (s2 + 1) * SLOT, :]  # [c, p]; i = c*128 + p
            flat = sub.reshape(-1)
            wrapped = flat.reshape(NIDX // 16, 16).T  # idx16[ch, col] = flat[col*16+ch]
            c0 = t * (NSUB * NIDX // 16) + s2 * (NIDX // 16)
            idx16[:, c0 : c0 + NIDX // 16] = wrapped
    idx_full = np.tile(idx16, (8, 1))
    dfm = np.asarray(d_c, dtype=np.float32).reshape(1, BC)
    return idx_full, mq, dfm


TRACE = False
LAST_RESULT = None


def kernel(**inputs):
    import ml_dtypes
    from concourse.bass_utils import run_bass_kernel_spmd

    nc = build()

    f32 = np.float32
    bf = ml_dtypes.bfloat16
    f8 = ml_dtypes.float8_e3m4
    emb = np.asarray(inputs["emb"], dtype=f32)
    emb4 = np.ascontiguousarray((emb.reshape(V4, E4) * ESCALE).astype(f8))
    x = np.asarray(inputs["x"], dtype=np.int64)
    d = np.asarray(inputs["d"], dtype=np.int64)

    Tb0 = np.asarray(inputs["Tb0"], f32)
    Tlb0 = np.asarray(inputs["Tlb0"], f32)
    Tb1 = np.asarray(inputs["Tb1"], f32)
    Tlb1 = np.asarray(inputs["Tlb1"], f32)
    Tk1 = np.asarray(inputs["Tk1"], f32)
    TA1 = np.asarray(inputs["TA1"], f32)
    TB1 = np.asarray(inputs["TB1"], f32)
    TB0 = np.asarray(inputs["TB0"], f32)
    TA0 = np.asarray(inputs["TA0"], f32)
    DA0 = np.asarray(inputs["DA0"], f32)
    DB0 = np.asarray(inputs["DB0"], f32)
    Dlb0 = np.asarray(inputs["Dlb0"], f32)
    h0W = np.asarray(inputs["h0W"], f32)
    h1W = np.asarray(inputs["h1W"], f32)
    h0b = np.asarray(inputs["h0b"], f32)
    h1b = np.asarray(inputs["h1b"], f32)

    pk1 = []
    for tt in range(2):
        pkt = np.zeros((128, 32), f32)
        pkt[:, 0:16] = TA1[tt]
        pkt[:, 16] = Tk1[:, 0]
        pk1.append(pkt)

    # rank-1 broadcast matrices
    FD1 = []
    LB1 = []
    for tt in range(2):
        fd = np.zeros((32, 128), f32)
        fd[0:16, :] = TB1[tt]  # (16,1) broadcast along cols
        fd[16, :] = 1.0
        FD1.append(fd)
        LB1.append(np.repeat(TB1[tt], 128, axis=1))
    K1b = np.repeat(Tk1, 128, axis=1)

    def c128(v):
        return np.full((128, 1), v, f32)

    def kmajor(W, k, dtype=f32):
        """Rearrange (k*128, N) so a row-major DMA into a (128, k, N) SBUF
        tile yields tile[p, j] = W[j*128 + p]."""
        W = np.asarray(W, f32)
        n = W.shape[1] if W.ndim > 1 else 1
        return np.ascontiguousarray(
            W.reshape(k, 128, n).transpose(1, 0, 2).reshape(k * 128, n)
        ).astype(dtype)

    G8sel = np.zeros((8, 8 * 128), f32)
    for r in range(8):
        G8sel[r, r * 128 : (r + 1) * 128] = 1.0

    diota = (np.arange(128) // 16).astype(f32).reshape(128, 1)

    shared = {
        "emb4": emb4,
        "sW0": kmajor(inputs["sW0"], 16, bf),
        "gW0": kmajor(inputs["gW0"], 16, bf),
        "sW1": kmajor(inputs["sW1"], 8, bf),
        "gW1": kmajor(inputs["gW1"], 8, bf),
        "sb0": kmajor(np.asarray(inputs["sb0"], f32).reshape(1024, 1), 8),
        "gb0": kmajor(np.asarray(inputs["gb0"], f32).reshape(1024, 1), 8),
        "sb1": kmajor(np.asarray(inputs["sb1"], f32).reshape(512, 1), 4),
        "gb1": kmajor(np.asarray(inputs["gb1"], f32).reshape(512, 1), 4),
        "Dk0": kmajor(inputs["Dk0"], 4, bf),
        "Db0": kmajor(np.asarray(inputs["Db0"], f32).reshape(256, 1), 2),
        "DA0p": kmajor(DA0.transpose(1, 0, 2).reshape(512, 128), 4, bf),
        "DB0p": np.ascontiguousarray(DB0.reshape(128, 256)).astype(bf),
        "Dlb0p": np.ascontiguousarray(Dlb0).astype(bf),
        "Tk0": kmajor(inputs["Tk0"], 2, bf),
        "TA0_0": kmajor(TA0[0], 2, bf),
        "TA0_1": kmajor(TA0[1], 2, bf),
        "TB0_0": np.ascontiguousarray(TB0[0]).astype(bf),
        "TB0_1": np.ascontiguousarray(TB0[1]).astype(bf),
        "bias_d0": (Tb0 + Tlb0[0]).reshape(128, 1),
        "bias_d1": (Tb0 + Tlb0[1]).reshape(128, 1),
        "bias_s": Tb0.reshape(128, 1),
        "bias_l0": Tlb0[0].reshape(128, 1),
        "bias_l1": Tlb0[1].reshape(128, 1),
        "pk1_0": pk1[0].astype(bf),
        "pk1_1": pk1[1].astype(bf),
        "TA1_0": np.ascontiguousarray(TA1[0]).astype(bf),
        "TA1_1": np.ascontiguousarray(TA1[1]).astype(bf),
        "FD1_0": FD1[0].astype(bf),
        "FD1_1": FD1[1].astype(bf),
        "LB1_0": LB1[0].astype(bf),
        "LB1_1": LB1[1].astype(bf),
        "K1b": K1b.astype(bf),
        "c_d0": c128(Tb1[0] + Tlb1[0, 0]),
        "c_d1": c128(Tb1[0] + Tlb1[1, 0]),
        "c_s": c128(Tb1[0]),
        "c_l0": c128(Tlb1[0, 0]),
        "c_l1": c128(Tlb1[1, 0]),
        "HWp": kmajor(np.concatenate([h0W, h1W], axis=1), 4, bf),
        "hb": np.concatenate([h0b, h1b]).reshape(8, 1),
        "onesr": np.ones((1, 128), f32),
        "diota": diota,
        "iota8": np.arange(8, dtype=f32).reshape(8, 1),
        "identb": np.eye(128, dtype=f32).astype(bf),
        "G8sel": G8sel.astype(bf),
    }

    in_maps = []
    for c in range(NCORES):
        idx_full, mq, dfm = _prep_core(
            x[c * BC : (c + 1) * BC], d[c * BC : (c + 1) * BC]
        )
        m = dict(shared)
        m["idxd"] = idx_full
        m["mqd"] = mq
        m["dfm"] = dfm
        in_maps.append(m)

    res = run_bass_kernel_spmd(nc, in_maps, list(range(NCORES)), trace=TRACE)
    global LAST_RESULT
    LAST_RESULT = res
    outs = [np.asarray(res.results[c]["out"]) for c in range(NCORES)]
    return np.concatenate(outs, axis=1).astype(np.float32)  # (2, B, 128)
